# revision 33
# baseline (speedup 1.0000x reference)
"""Cross-attention kernel for Trainium2 (8 NeuronCores, SPMD data-parallel).

Problem: O = softmax(Q @ K^T) @ V with B=4, Lq=Lk=4096, D=64, fp32 (no
1/sqrt(d) scaling).

Sharding: 8 cores = 4 batches x 2 Lq-halves. Each core handles a
[2048, 64] Q shard against the full [4096, 64] K/V of its batch.
Independent outputs -> no collectives.

Per-core algorithm (layouts chosen so nothing is transposed on-chip):
  - Host supplies QT/KT as piece-major fp16 [n_pieces, 64, 512] (contiguous
    pieces DMA into large descriptors; strided layouts fragment into 8-row
    descriptors) and VA partition-major bf16 [128, 32, 128] = [V|ones|0]
    chunks.  On-chip tiles are 128 partitions tall with rows 64..127 of
    QT/KT memset to zero.
  - The zero padding makes every matmul a full 128x128-activity op.
    TRN2's PE_HAM clock gate only un-throttles (1.2 -> 2.4 GHz) when the
    PE array's activity is high; half-array matmuls (contraction 64, or
    65 output partitions) never cross the threshold and the whole kernel
    runs at half clock. Padded operands cost the same cycles (cycles =
    moving-dim size) but register full activity -> warm clock.
  - ST[k, q] = matmul(lhsT=KTpad chunk [128,128], rhs=QT [128,512]);
    rows 64..127 of KTpad are zero so the pad QT rows contribute 0.
  - PT = exp(ST) on the scalar engine, written as bf16 (no max
    subtraction: |scores| < ~50, exp fits fp32/bf16 range; fp16 P would
    underflow). The scalar engine at 1 elem/cycle/lane (~64.3us busy) is
    the kernel's roofline; everything else hides under it.  Exp
    instructions are kept at 1024 free elements (st bufs=3 removes the
    WAR bubble between exp and the +2-chunk score matmul).
  - OT[128, q] += matmul(lhsT=VA chunk [128, 128] bf16, rhs=PT [128, 512]):
    rows 0..63 accumulate unnormalized output, row 64 the softmax
    denominator, rows 65..127 zeros. PT is consumed directly as rhs -
    no transpose anywhere.
  - Normalize: den-copy to SBUF (custom DVE ops misread PSUM), DVE
    fast-approx reciprocal (~5e-4, no NR pass needed), gpsimd partition-
    broadcast, multiply, DMA out OT [64, 2048]; host transposes back.
    The single OT buffer frees 2 PSUM banks for st bufs=3; qb1's first
    HOIST chunks are emitted score+exp-only so the in-order PE queue
    keeps feeding the scalar engine while qb0's normalization drains.
"""

import sys

for _p in ("/opt/trn_rl_repo", "/opt/pypackages"):
    if _p not in sys.path:
        sys.path.insert(0, _p)

from contextlib import ExitStack

import ml_dtypes
import numpy as np

import concourse.bacc as bacc
import concourse.mybir as mybir
import concourse.tile as tile
from concourse.bass_utils import run_bass_kernel_spmd

# Problem constants (hardcoded per contract).
B, LQ, LK, D = 4, 4096, 4096, 64
N_CORES = 8
LQ_SHARD = LQ * B // N_CORES  # 2048
QB = 1024  # q-block (exp instruction free-size; 2 PSUM banks)
NQB = LQ_SHARD // QB  # 2
KC = 128  # k-chunk (contraction tile for the PV matmul)
NKC = LK // KC  # 32
SL = 512  # matmul moving-dim slice (one PSUM bank)
NSL = QB // SL  # 2

F32 = mybir.dt.float32
F16 = mybir.dt.float16
BF16 = mybir.dt.bfloat16

BF16NP = ml_dtypes.bfloat16

KT_PIECE = 512  # kt DMA piece width (cols); 4 k-chunks per piece
VA_PIECE = 8  # va DMA piece size in k-chunks


def _build_program():
    nc = bacc.Bacc(
        "TRN2",
        target_bir_lowering=False,
        debug=False,
        num_devices=N_CORES,
    )
    # QT/KT piece-major: [piece, 64, 512] contiguous so each piece DMA
    # merges into large descriptors (strided [128, 4096] layouts fragment
    # into 8-row descriptors at ~300ns cadence).  Only the 64 real rows
    # move; rows 64..127 of the SBUF tiles are memset to zero on-chip.
    qt_d = nc.declare_dram_parameter(
        "QT", [LQ_SHARD // SL, D, SL], F16, isOutput=False
    )
    kt_d = nc.declare_dram_parameter(
        "KT", [LK // KT_PIECE, D, KT_PIECE], F16, isOutput=False
    )
    # VA partition-major [p, c, d]: per-partition lines are contiguous 2KB+
    # (the natural [k, d] layout would scatter 256B lines and choke the DMA
    # queues for ~12us).
    va_d = nc.declare_dram_parameter("VA", [KC, NKC, KC], BF16, isOutput=False)
    ot_d = nc.declare_dram_parameter("OT", [D, LQ_SHARD], F32, isOutput=True)

    with tile.TileContext(nc) as tc, ExitStack() as ctx:
        singles = ctx.enter_context(tc.tile_pool(name="singles", bufs=1))
        st_pool = ctx.enter_context(tc.tile_pool(name="st", bufs=3, space="PSUM"))
        ot_pool = ctx.enter_context(tc.tile_pool(name="ot", bufs=1, space="PSUM"))
        pt_pool = ctx.enter_context(tc.tile_pool(name="pt", bufs=8))
        out_pool = ctx.enter_context(tc.tile_pool(name="out", bufs=2))
        norm_pool = ctx.enter_context(tc.tile_pool(name="norm", bufs=4))

        # Preload the exp activation table while input DMAs run.
        warm = singles.tile([1, 2], F32)
        nc.vector.memset(warm[:, :], 0.0)
        nc.scalar.activation(
            out=warm[:, :], in_=warm[:, :],
            func=mybir.ActivationFunctionType.Exp,
        )

        # Inputs are split into pieces so the first score matmuls don't
        # wait for the full 2.5 MB of loads.
        KH = LK // 2  # kt half width
        VH = NKC // 2  # va half size in chunks
        qt_sb = [singles.tile([2 * D, QB], F16, name=f"qt{h}") for h in range(2)]
        kt_sb = [singles.tile([2 * D, KH], F16, name=f"kt{h}") for h in range(2)]
        va_sb = [
            singles.tile([KC, VH, KC], BF16, name=f"va{h}") for h in range(2)
        ]

        # Zero the pad rows (matmul weights/rhs rows 64..127) on the DVE
        # while the DMAs run; zero weights make those rows contribute 0.
        # qt0's pad gates the very first matmul, so it goes first.
        nc.vector.memset(qt_sb[0][D : 2 * D, :], 0.0)
        nc.vector.memset(kt_sb[0][D : 2 * D, :], 0.0)
        nc.vector.memset(kt_sb[1][D : 2 * D, :], 0.0)
        nc.vector.memset(qt_sb[1][D : 2 * D, :], 0.0)

        def dma_qt(h, p):
            sl = slice(p * SL, (p + 1) * SL)
            nc.sync.dma_start(
                out=qt_sb[h][0:D, sl], in_=qt_d[h * (QB // SL) + p, :, :]
            )

        def dma_kt(h, p):
            sl = slice(p * KT_PIECE, (p + 1) * KT_PIECE)
            nc.sync.dma_start(
                out=kt_sb[h][0:D, sl],
                in_=kt_d[h * (KH // KT_PIECE) + p, :, :],
            )

        def dma_va(h, p):
            sl = slice(p * VA_PIECE, (p + 1) * VA_PIECE)
            sg = slice(h * VH + p * VA_PIECE, h * VH + (p + 1) * VA_PIECE)
            nc.sync.dma_start(out=va_sb[h][:, sl, :], in_=va_d[:, sg, :])

        # Enqueue order = criticality: the Sync engine needs ~600ns per
        # dma_start and downstream waits cover every DMA enqueued earlier,
        # so the first chunk's score inputs (kt0 p0, both qt0 pieces) go
        # first and everything for the second q/k half goes last.
        dma_kt(0, 0)
        dma_qt(0, 0)
        dma_qt(0, 1)
        dma_va(0, 0)
        for p in range(1, KH // KT_PIECE):
            dma_kt(0, p)
        dma_va(0, 1)
        for p in range(KH // KT_PIECE):
            dma_kt(1, p)
        for p in range(VH // VA_PIECE):
            dma_va(1, p)
        dma_qt(1, 0)
        dma_qt(1, 1)

        def kt_ap(c):
            # [128, 128] fp16 weights for chunk c (rows 64..127 zero)
            t = kt_sb[c * KC // KH]
            off = (c * KC) % KH
            return t[:, off : off + KC]

        def va_ap(c):
            return va_sb[c // VH][:, c % VH, :]

        def emit_score_exp(qb, c):
            qt = qt_sb[qb]
            st_ps = st_pool.tile([KC, QB], F32, tag="st")
            pt = pt_pool.tile([KC, QB], BF16)
            for s in range(NSL):
                nc.tensor.matmul(
                    out=st_ps[:, s * SL : (s + 1) * SL],
                    lhsT=kt_ap(c),
                    rhs=qt[:, s * SL : (s + 1) * SL],
                    start=True,
                    stop=True,
                )
            nc.scalar.activation(
                out=pt[:, :],
                in_=st_ps[:, :],
                func=mybir.ActivationFunctionType.Exp,
            )
            return pt

        def emit_pv(ot_ps, c, pt):
            for s in range(NSL):
                nc.tensor.matmul(
                    out=ot_ps[:, s * SL : (s + 1) * SL],
                    lhsT=va_ap(c),
                    rhs=pt[:, s * SL : (s + 1) * SL],
                    start=(c == 0),
                    stop=(c == NKC - 1),
                )

        def emit_norm(qb, ot_ps, act_copy=False):
            # Normalize: O[d, q] = OT[d, q] / OT[64, q].  The fast-approx
            # reciprocal alone is ~5e-4 relative - far below the bf16 P
            # error - so no Newton-Raphson pass.
            recips = []
            for s in range(NSL):
                sl = slice(s * SL, (s + 1) * SL)
                den = norm_pool.tile([1, SL], F32)
                if act_copy:
                    # Tail norm: the scalar engine's queue is empty after
                    # the last exp, so the PSUM->SBUF den copies run there
                    # in parallel with the DVE reciprocals.
                    nc.scalar.activation(
                        out=den[:, :],
                        in_=ot_ps[D : D + 1, sl],
                        func=mybir.ActivationFunctionType.Copy,
                    )
                else:
                    nc.vector.tensor_copy(den[:, :], ot_ps[D : D + 1, sl])
                recip = norm_pool.tile([1, SL], F32)
                nc.vector.reciprocal_approx_fast(
                    out=recip[:, :], in_=den[:, :]
                )
                recips.append(recip)
            bcasts = []
            for s in range(NSL):
                bcast = norm_pool.tile([D, SL], F32)
                nc.gpsimd.partition_broadcast(bcast[:, :], recips[s][:, :])
                bcasts.append(bcast)
            for s in range(NSL):
                sl = slice(s * SL, (s + 1) * SL)
                o_sb = out_pool.tile([D, SL], F32)
                nc.vector.tensor_mul(
                    o_sb[:, :], ot_ps[0:D, sl], bcasts[s][:, :]
                )
                nc.sync.dma_start(
                    out=ot_d[:, qb * QB + s * SL : qb * QB + (s + 1) * SL],
                    in_=o_sb[:, :],
                )

        # qb 0: straightforward chunk loop.
        ot0 = ot_pool.tile([KC, QB], F32, tag="ot")
        for c in range(NKC):
            pt = emit_score_exp(0, c)
            emit_pv(ot0, c, pt)
        emit_norm(0, ot0)

        # qb 1: hoist the first HOIST chunks' scores+exps ahead of their
        # PV matmuls.  With a single OT buffer, qb1's first PV must wait
        # for qb0's normalization to finish reading OT; hoisting keeps the
        # scalar engine streaming exps through that window (the PE queue
        # is in-order, so anything behind the blocked PV would stall too).
        HOIST = 6
        ot1 = ot_pool.tile([KC, QB], F32, tag="ot")
        pend = []  # (c, pt) with PV not yet emitted
        for c in range(HOIST):
            pend.append((c, emit_score_exp(1, c)))
        for c in range(HOIST, NKC):
            pt = emit_score_exp(1, c)
            pend.append((c, pt))
            cc, pp = pend.pop(0)
            emit_pv(ot1, cc, pp)
            if len(pend) > 1 and c < 2 * HOIST:  # catch the lag back up
                cc, pp = pend.pop(0)
                emit_pv(ot1, cc, pp)
        for cc, pp in pend:
            emit_pv(ot1, cc, pp)
        emit_norm(1, ot1, act_copy=True)

    nc.finalize()
    return nc


_PROGRAM_CACHE = {}


def _get_program():
    if "nc" not in _PROGRAM_CACHE:
        _PROGRAM_CACHE["nc"] = _build_program()
    return _PROGRAM_CACHE["nc"]


def _make_in_maps(Q, K, V):
    Q = np.asarray(Q, dtype=np.float32)
    K = np.asarray(K, dtype=np.float32)
    V = np.asarray(V, dtype=np.float32)
    in_maps = []
    for core in range(N_CORES):
        b, half = core // 2, core % 2
        q_shard = Q[b, half * LQ_SHARD : (half + 1) * LQ_SHARD, :]  # [2048, 64]
        qt1 = q_shard.T.astype(np.float16)  # [64, 2048]
        # piece-major [n_pieces, 64, 512]
        qt = qt1.reshape(D, LQ_SHARD // 512, 512).transpose(1, 0, 2)
        kt1 = K[b].T.astype(np.float16)  # [64, 4096]
        kt = kt1.reshape(D, LK // 512, 512).transpose(1, 0, 2)
        va = np.zeros((LK, KC), dtype=BF16NP)  # [4096, 128]
        va[:, :D] = V[b].astype(BF16NP)
        va[:, D] = 1.0
        # partition-major [p, c, d] so device DMA lines are contiguous
        va_pm = va.reshape(NKC, KC, KC).transpose(1, 0, 2)
        in_maps.append(
            {
                "QT": np.ascontiguousarray(qt),
                "KT": np.ascontiguousarray(kt),
                "VA": np.ascontiguousarray(va_pm),
            }
        )
    return in_maps


def _run(Q, K, V, trace=False, **spmd_kwargs):
    nc = _get_program()
    in_maps = _make_in_maps(Q, K, V)
    res = run_bass_kernel_spmd(
        nc, in_maps, list(range(N_CORES)), trace=trace, **spmd_kwargs
    )
    out = np.empty((B, LQ, D), dtype=np.float32)
    for core in range(N_CORES):
        b, half = core // 2, core % 2
        ot = res.results[core]["OT"]  # [64, 2048]
        out[b, half * LQ_SHARD : (half + 1) * LQ_SHARD, :] = ot.T
    return out, res


def kernel(Q, K, V):
    out, _ = _run(Q, K, V, trace=False)
    return out
